# revision 26
# baseline (speedup 1.0000x reference)
"""Trainium2 Bass kernel for a pre-norm MQA decoder layer (dense_transformer).

Model (per batch element b, seq s=2048, d=4096, 32 heads x dk=128, d_ff=16384):
  xn = rmsnorm(x)*scale_attn; q,k,v = proj(xn) (MQA: single k/v head)
  attn = softmax(q k^T / sqrt(dk)) v;  x1 = x + attn @ Wo + bo
  xn2 = rmsnorm(x1)*scale_ffn;  out = x1 + gelu(xn2 @ W1 + b1) @ W2 + b2

Sharding: pure data parallel over 8 cores. Each core owns 512 query tokens
(batch be=c//4, rows (c%4)*512..+512) and redundantly computes the full
2048-token K/V for its batch element (cheap for MQA: dk=128). No collectives.
Per-core kv input is rotated so the core's own 512 tokens are always slab 0
(softmax is permutation-invariant over kv).

Everything on device is feature-major [d on partitions, tokens on free]: the
host ships x pre-transposed ([D, S] bf16) and transposes the [D, T] f32
output back, so the kernel has no PE transposes except 16 tiny V-tiles.
Per-token rmsnorm stats are partition reductions done as ones-vector matmuls,
then broadcast back with a K=1 matmul; reciprocals use the fast approx DVE op.
All matmul operands are bf16 (separate pulled-ahead LDWEIGHTS + FWL; f32/f32r
would self-load the stationary inside MATMUL at ~2.2x the cost). Accumulation
is f32 in PSUM; the residual stream x1 is f32 in SBUF.

Engines execute in program order, so the program is laid out to keep the PE
dense (HAM re-throttles to 1.2GHz after ~3.4us of idle):
 - Q-projection m-groups are interleaved between the four K/V token groups,
   covering each group's DMA/normalize latency with 27us of Q matmuls.
 - Attention issues score matmuls one kv-pair ahead and exponentiates two
   512-wide chunks per Activation instruction, so the sum/attn matmuls never
   wait on the scalar engine.
 - GEMM sections get 6-8 PSUM banks so consecutive accumulator blocks
   overlap; cheap DVE work (x1 init, rmsnorm2 stats) is interleaved where the
   PE would otherwise wait.
"""

import sys

if "/opt/trn_rl_repo" not in sys.path:
    sys.path.insert(0, "/opt/trn_rl_repo")

import numpy as np

P = 128
T = 512            # tokens per core
D = 4096
DC = D // P        # 32 feature chunks
DK = 128
NH = 32
S = 2048           # kv length
SC = S // P        # 16 kv chunks
SP = SC // 2       # kv chunk pairs
DFF = 16384
FC = DFF // P      # 128 ff chunks
FBLK = 1024        # FFN f-block width
NFB = DFF // FBLK  # 16 f-blocks
NCORES = 8
EPS = 1e-10
KSCALE = 1.0 / float(np.sqrt(128.0))

_CACHE = {}
LAST_RESULTS = None  # test.py reads exec_time_ns from here


def _build_program():
    import concourse.tile as tile
    from concourse import bacc, mybir
    from concourse.masks import make_identity

    f32 = mybir.dt.float32
    bf16 = mybir.dt.bfloat16
    AF = mybir.ActivationFunctionType
    ALU = mybir.AluOpType

    nc = bacc.Bacc("TRN2", target_bir_lowering=False, num_devices=NCORES)

    x_t = nc.dram_tensor("x_t", [D, S], bf16, kind="ExternalInput")
    wq = nc.dram_tensor("wq", [D, D], bf16, kind="ExternalInput")
    wk = nc.dram_tensor("wk", [D, DK], bf16, kind="ExternalInput")
    wv = nc.dram_tensor("wv", [D, DK], bf16, kind="ExternalInput")
    wo = nc.dram_tensor("wo", [D, D], bf16, kind="ExternalInput")
    w1 = nc.dram_tensor("w1", [D, DFF], bf16, kind="ExternalInput")
    w2 = nc.dram_tensor("w2", [DFF, D], bf16, kind="ExternalInput")
    bq = nc.dram_tensor("bq", [D], f32, kind="ExternalInput")
    bk = nc.dram_tensor("bk", [DK], f32, kind="ExternalInput")
    bv = nc.dram_tensor("bv", [DK], f32, kind="ExternalInput")
    bo = nc.dram_tensor("bo", [D], f32, kind="ExternalInput")
    b1 = nc.dram_tensor("b1", [DFF], f32, kind="ExternalInput")
    b2 = nc.dram_tensor("b2", [D], f32, kind="ExternalInput")
    out_t = nc.dram_tensor("out_t", [D, T], f32, kind="ExternalOutput")

    lowp = nc.allow_low_precision(
        reason="bf16 matmul operands are the intended precision here")
    with lowp, tile.TileContext(nc) as tc:
        consts = tc.alloc_tile_pool(name="consts", bufs=1)
        ident = consts.tile([P, P], f32)
        make_identity(nc, ident)
        ident_b = consts.tile([P, P], bf16)
        nc.vector.tensor_copy(ident_b, ident)
        ones_f = consts.tile([P, 1], f32)
        nc.vector.memset(ones_f, 1.0)
        ones_col = consts.tile([P, 1], bf16)
        nc.vector.tensor_copy(ones_col, ones_f)
        ones_rf = consts.tile([1, P], f32)
        nc.vector.memset(ones_rf, 1.0)
        ones_row = consts.tile([1, P], bf16)
        nc.vector.tensor_copy(ones_row, ones_rf)
        eps_sb = consts.tile([P, 1], f32)
        nc.vector.memset(eps_sb, EPS)
        bq_sb = consts.tile([P, DC], f32)
        nc.sync.dma_start(bq_sb, bq[:].rearrange("(c p) -> p c", p=P))
        bo_sb = consts.tile([P, DC], f32)
        nc.sync.dma_start(bo_sb, bo[:].rearrange("(c p) -> p c", p=P))
        b2_sb = consts.tile([P, DC], f32)
        nc.sync.dma_start(b2_sb, b2[:].rearrange("(c p) -> p c", p=P))
        b1_sb = consts.tile([P, FC], f32)
        nc.sync.dma_start(b1_sb, b1[:].rearrange("(c p) -> p c", p=P))
        bk_sb = consts.tile([P, 1], f32)
        nc.sync.dma_start(bk_sb, bk[:][:, None])
        bv_sb = consts.tile([P, 1], f32)
        nc.sync.dma_start(bv_sb, bv[:][:, None])

        nb2 = consts.tile([P, T], bf16)

        # long-lived activation buffers (never released; FFN fits alongside)
        kv_out = tc.alloc_tile_pool(name="kv_out", bufs=1)
        kT = kv_out.tile([P, S], bf16)          # k^T: dk on partitions
        vtok = kv_out.tile([P, SC, DK], bf16)   # v token-major kv chunks
        p_head = tc.alloc_tile_pool(name="p_head", bufs=32)

        # ---- Phase 1+2: K/V token groups with Q-projection m-groups
        # interleaved to keep the PE dense while group DMAs/norms land.
        q_tiles = []
        with (
            tc.tile_pool(name="p_xn", bufs=1) as p_xn,
            tc.tile_pool(name="gchunk", bufs=44) as gchunk_p,
            tc.tile_pool(name="sqp", bufs=2) as sq_p,
            tc.tile_pool(name="wkv", bufs=1) as wkv_p,
            tc.tile_pool(name="nstat", bufs=3) as nstat_p,
            tc.tile_pool(name="nbp", bufs=2) as nb_p,
            tc.tile_pool(name="vtmp", bufs=2) as vtmp_p,
            tc.tile_pool(name="wq_s", bufs=48) as wq_p,
            tc.tile_pool(name="ps_misc", bufs=2, space="PSUM") as ps_misc,
            tc.tile_pool(name="ps_kv", bufs=2, space="PSUM") as ps_kv,
            tc.tile_pool(name="ps_tr", bufs=1, space="PSUM") as ps_tr,
            tc.tile_pool(name="ps_q", bufs=3, space="PSUM") as ps_q,
        ):
            xnT = p_xn.tile([P, DC, T], bf16)
            # K/V weights: load once, reuse across all 4 token groups
            wk_sb = wkv_p.tile([P, DC, DK], bf16)
            wv_sb = wkv_p.tile([P, DC, DK], bf16)
            nc.sync.dma_start(wk_sb, wk[:].rearrange("(c p) k -> p c k", p=P))
            nc.sync.dma_start(wv_sb, wv[:].rearrange("(c p) k -> p c k", p=P))

            def do_group(g):
                chunks = []
                ssum = ps_misc.tile([1, T], f32, tag="misc", name=f"ss{g}")
                for c in range(DC):
                    if g == 0:
                        ch = xnT[:, c, :]
                    else:
                        ch = gchunk_p.tile([P, T], bf16, tag="ch")
                    nc.sync.dma_start(
                        ch, x_t[c * P:(c + 1) * P, g * T:(g + 1) * T])
                    chunks.append(ch)
                    sq = sq_p.tile([P, T], bf16, tag="sq")
                    nc.vector.tensor_mul(sq, ch, ch)
                    nc.tensor.matmul(ssum, ones_col, sq,
                                     start=(c == 0), stop=(c == DC - 1))
                rms = nstat_p.tile([1, T], f32, tag="rms")
                nc.scalar.activation(rms, ssum, AF.Sqrt, bias=eps_sb[:1, 0:1],
                                     scale=1.0 / D)
                inv32 = nstat_p.tile([1, T], f32, tag="inv32")
                nc.vector.reciprocal_approx_fast(out=inv32, in_=rms)
                inv16 = nstat_p.tile([1, T], bf16, tag="inv16")
                nc.vector.tensor_copy(inv16, inv32)
                nbg_ps = ps_misc.tile([P, T], f32, tag="misc", name=f"nb{g}")
                nc.tensor.matmul(nbg_ps, ones_row, inv16, start=True, stop=True)
                nbg = nb_p.tile([P, T], bf16, tag="nbg")
                nc.vector.tensor_copy(nbg, nbg_ps)

                kps = ps_kv.tile([P, T], f32, tag="kv", name=f"kps{g}")
                vps = ps_kv.tile([P, T], f32, tag="kv", name=f"vps{g}")
                for c in range(DC):
                    nc.vector.tensor_mul(chunks[c], chunks[c], nbg)
                    nc.tensor.matmul(kps, wk_sb[:, c, :], chunks[c],
                                     start=(c == 0), stop=(c == DC - 1))
                    nc.tensor.matmul(vps, wv_sb[:, c, :], chunks[c],
                                     start=(c == 0), stop=(c == DC - 1))
                nc.scalar.activation(kT[:, g * T:(g + 1) * T], kps, AF.Identity,
                                     bias=bk_sb[:, 0:1])
                vt = vtmp_p.tile([P, T], bf16, tag="vt")
                nc.scalar.activation(vt, vps, AF.Identity, bias=bv_sb[:, 0:1])
                for q4 in range(4):
                    pt = ps_tr.tile([P, P], bf16, tag="tr")
                    nc.tensor.transpose(pt, vt[:, q4 * P:(q4 + 1) * P], ident_b)
                    nc.vector.tensor_copy(vtok[:, g * 4 + q4, :], pt)

            def do_qmg(mg):
                # j-serialized: one PSUM accumulator at a time, the 32 weight
                # tiles of this m-group stay resident across the 4 j-passes
                wbs = []
                for kc in range(DC):
                    wb = wq_p.tile([P, 512], bf16, tag="wq")
                    nc.sync.dma_start(
                        wb, wq[kc * P:(kc + 1) * P, mg * 512:(mg + 1) * 512])
                    wbs.append(wb)
                for j in range(4):
                    m = mg * 4 + j
                    psq = ps_q.tile([P, T], f32, tag="q", name=f"psq{m}")
                    for kc in range(DC):
                        nc.tensor.matmul(psq, wbs[kc][:, j * P:(j + 1) * P],
                                         xnT[:, kc, :],
                                         start=(kc == 0), stop=(kc == DC - 1))
                    qt = p_head.tile([P, T], bf16, tag="head", name=f"q{m}")
                    nc.scalar.activation(qt, psq, AF.Identity,
                                         bias=bq_sb[:, m:m + 1])
                    q_tiles.append(qt)

            do_group(0)
            do_qmg(0)
            do_qmg(1)
            do_group(1)
            do_qmg(2)
            do_qmg(3)
            do_group(2)
            do_qmg(4)
            do_qmg(5)
            do_group(3)
            do_qmg(6)
            do_qmg(7)

        # x1T: f32 residual-stream accumulator (allocated only now; phase 1+2
        # needed the SBUF for resident weight tiles)
        p_x1 = tc.alloc_tile_pool(name="p_x1", bufs=1)
        x1T = p_x1.tile([P, DC, T], f32)

        # ---- Phase 3+4: attention; x1T init (raw own x^T + bo) interleaved
        # one chunk per head on DMA+DVE. Attention output for head h
        # overwrites q_tiles[h] in place. Scores run one kv-pair ahead and
        # exp handles two 512-chunks per instruction.
        with (
            tc.tile_pool(name="xrp", bufs=4) as xr_p,
            tc.tile_pool(name="expp", bufs=4) as exp_p,
            tc.tile_pool(name="atsb", bufs=32) as at_sb_p,
            tc.tile_pool(name="bcp", bufs=2) as bc_p,
            tc.tile_pool(name="smalls", bufs=4) as small_p,
            tc.tile_pool(name="recp", bufs=1) as rec_p,
            tc.tile_pool(name="ps_scp", bufs=2, space="PSUM") as ps_scp,
            tc.tile_pool(name="ps_sum", bufs=2, space="PSUM") as ps_sum,
            tc.tile_pool(name="ps_at", bufs=2, space="PSUM") as ps_at,
        ):
            at_tiles = []
            rec_all = rec_p.tile([1, NH, T], bf16)
            for h in range(NH):
                # x1T chunk h init: raw own x^T + bo (no PE work)
                xr = xr_p.tile([P, T], bf16, tag="xr", name=f"xr{h}")
                nc.sync.dma_start(xr, x_t[h * P:(h + 1) * P, 0:T])
                nc.vector.tensor_scalar_add(x1T[:, h, :], xr, bo_sb[:, h:h + 1])

                sum_ps = ps_sum.tile([1, T], f32, tag="sum", name=f"sum{h}")
                at_ps = ps_at.tile([P, T], f32, tag="at", name=f"at{h}")
                ex_pairs = []

                def issue_scores(pp):
                    # score pairs run two pairs ahead of sum/at so the PE
                    # never waits on the scalar engine's exp; one exp covers
                    # both 512-chunks (fewer ACT instructions + sem waits)
                    pr = ps_scp.tile([P, 2, T], f32, tag="sc",
                                     name=f"sc{h}_{pp}")
                    nc.tensor.matmul(pr[:, 0, :],
                                     kT[:, (2 * pp) * P:(2 * pp + 1) * P],
                                     q_tiles[h], start=True, stop=True)
                    nc.tensor.matmul(pr[:, 1, :],
                                     kT[:, (2 * pp + 1) * P:(2 * pp + 2) * P],
                                     q_tiles[h], start=True, stop=True)
                    ex = exp_p.tile([P, 2, T], bf16, tag="ex",
                                    name=f"ex{h}_{pp}")
                    nc.scalar.activation(ex, pr, AF.Exp, scale=KSCALE)
                    ex_pairs.append(ex)

                issue_scores(0)
                issue_scores(1)
                for pp in range(SP):
                    ex = ex_pairs[pp]
                    for half in range(2):
                        sc = 2 * pp + half
                        nc.tensor.matmul(sum_ps, ones_col, ex[:, half, :],
                                         start=(sc == 0), stop=(sc == SC - 1))
                        nc.tensor.matmul(at_ps, vtok[:, sc, :], ex[:, half, :],
                                         start=(sc == 0), stop=(sc == SC - 1))
                    if pp + 2 < SP:
                        issue_scores(pp + 2)
                # move at off PSUM immediately (frees the bank for head h+2);
                # per-head normalization is batched after the loop
                rec32 = small_p.tile([1, T], f32, tag="rec32", name=f"rec32_{h}")
                nc.vector.reciprocal_approx_fast(out=rec32, in_=sum_ps)
                nc.vector.tensor_copy(rec_all[:, h, :], rec32)
                at_sb = at_sb_p.tile([P, T], bf16, tag="at", name=f"atsb{h}")
                nc.vector.tensor_copy(at_sb, at_ps)
                at_tiles.append(at_sb)

            for h in range(NH):
                bc_ps = ps_at.tile([P, T], f32, tag="at", name=f"bc{h}")
                nc.tensor.matmul(bc_ps, ones_row, rec_all[:, h, :],
                                 start=True, stop=True)
                bc = bc_p.tile([P, T], f32, tag="bc", name=f"bcs{h}")
                nc.vector.tensor_copy(bc, bc_ps)
                nc.vector.tensor_mul(q_tiles[h], at_tiles[h], bc)
        attn_tiles = q_tiles

        # ---- Phase 5: Wo + residual into x1T; rmsnorm2 stats (sq + ones-
        # matmul) interleaved per jg block right after each chunk finalizes.
        with (
            tc.tile_pool(name="wo_s", bufs=8) as wo_p,
            tc.tile_pool(name="sq2", bufs=8) as sq2_p,
            tc.tile_pool(name="smalls2", bufs=2) as small2_p,
            tc.tile_pool(name="ps_wo", bufs=6, space="PSUM") as ps_wo,
            tc.tile_pool(name="ps_ss2", bufs=1, space="PSUM") as ps_ss2,
            tc.tile_pool(name="ps_nb2", bufs=1, space="PSUM") as ps_nb2,
        ):
            ssum2 = ps_ss2.tile([1, T], f32, tag="ss2")
            pend_sq = []
            for jg in range(8):
                pss = [ps_wo.tile([P, T], f32, tag="wo", name=f"pswo{jg}_{j}")
                       for j in range(4)]
                for kc in range(DC):
                    if kc == 8 and pend_sq:
                        # rmsnorm2 partial sums for the previous jg's chunks:
                        # emitted mid-block so their DVE inputs are ready
                        for c, sq in pend_sq:
                            nc.tensor.matmul(ssum2, ones_col, sq,
                                             start=(c == 0), stop=False)
                        pend_sq = []
                    wb = wo_p.tile([P, 512], bf16, tag="wob")
                    nc.sync.dma_start(wb, wo[kc * P:(kc + 1) * P, jg * 512:(jg + 1) * 512])
                    for j in range(4):
                        nc.tensor.matmul(pss[j], wb[:, j * P:(j + 1) * P],
                                         attn_tiles[kc],
                                         start=(kc == 0), stop=(kc == DC - 1))
                for j in range(4):
                    c = jg * 4 + j
                    nc.vector.tensor_tensor(x1T[:, c, :], pss[j], x1T[:, c, :], ALU.add)
                    sq = sq2_p.tile([P, T], bf16, tag="sq2", name=f"sq2_{c}")
                    nc.vector.tensor_mul(sq, x1T[:, c, :], x1T[:, c, :])
                    pend_sq.append((c, sq))
            for c, sq in pend_sq:
                nc.tensor.matmul(ssum2, ones_col, sq,
                                 start=False, stop=(c == DC - 1))
            rms2 = small2_p.tile([1, T], f32, tag="rms2")
            nc.scalar.activation(rms2, ssum2, AF.Sqrt, bias=eps_sb[:1, 0:1],
                                 scale=1.0 / D)
            inv2 = small2_p.tile([1, T], f32, tag="inv2")
            nc.vector.reciprocal_approx_fast(out=inv2, in_=rms2)
            inv2b = small2_p.tile([1, T], bf16, tag="inv2b")
            nc.vector.tensor_copy(inv2b, inv2)
            nb_ps = ps_nb2.tile([P, T], f32, tag="nb2")
            nc.tensor.matmul(nb_ps, ones_row, inv2b, start=True, stop=True)
            nc.vector.tensor_copy(nb2, nb_ps)

        # ---- Phase 6: FFN, f-blocked; W2 accumulated into x1T in place.
        # Last f-block fuses +b2 and streams the finished chunk out to DRAM.
        p_xn2 = tc.alloc_tile_pool(name="p_xn2", bufs=1)
        xn2T = p_xn2.tile([P, DC, T], bf16)
        for c in range(DC):
            nc.vector.tensor_mul(xn2T[:, c, :], x1T[:, c, :], nb2)

        MGS = FBLK // 512          # m-groups per f-block
        FCB = FBLK // P            # f chunks per f-block
        ALU_add = ALU.add
        with (
            tc.tile_pool(name="wf_s", bufs=8) as wf_p,
            tc.tile_pool(name="htp", bufs=20) as ht_p,
            tc.tile_pool(name="ps_ffn", bufs=8, space="PSUM") as ps_ffn,
        ):
            for fb in range(NFB):
                last_fb = fb == NFB - 1
                ht_tiles = []
                for mg in range(MGS):
                    pss = [ps_ffn.tile([P, T], f32, tag="ffn", name=f"psw1_{fb}_{mg}_{j}")
                           for j in range(4)]
                    for kc in range(DC):
                        wb = wf_p.tile([P, 512], bf16, tag="wf")
                        nc.sync.dma_start(
                            wb, w1[kc * P:(kc + 1) * P,
                                   fb * FBLK + mg * 512:fb * FBLK + (mg + 1) * 512])
                        for j in range(4):
                            nc.tensor.matmul(pss[j], wb[:, j * P:(j + 1) * P],
                                             xn2T[:, kc, :],
                                             start=(kc == 0), stop=(kc == DC - 1))
                    for j in range(4):
                        m = fb * FCB + mg * 4 + j
                        ht = ht_p.tile([P, T], bf16, tag="ht", name=f"ht{m}")
                        nc.scalar.activation(ht, pss[j], AF.Gelu, bias=b1_sb[:, m:m + 1])
                        ht_tiles.append(ht)
                for jg in range(8):
                    pss = [ps_ffn.tile([P, T], f32, tag="ffn", name=f"psw2_{fb}_{jg}_{j}")
                           for j in range(4)]
                    for fc in range(FCB):
                        wb = wf_p.tile([P, 512], bf16, tag="wf")
                        nc.sync.dma_start(
                            wb, w2[fb * FBLK + fc * P:fb * FBLK + (fc + 1) * P,
                                   jg * 512:(jg + 1) * 512])
                        for j in range(4):
                            nc.tensor.matmul(pss[j], wb[:, j * P:(j + 1) * P],
                                             ht_tiles[fc],
                                             start=(fc == 0), stop=(fc == FCB - 1))
                    for j in range(4):
                        c = jg * 4 + j
                        if last_fb:
                            # x1 += pss + b2, then stream the chunk out
                            nc.vector.scalar_tensor_tensor(
                                out=x1T[:, c, :], in0=pss[j],
                                scalar=b2_sb[:, c:c + 1], in1=x1T[:, c, :],
                                op0=ALU_add, op1=ALU_add)
                            nc.sync.dma_start(out_t[c * P:(c + 1) * P, :],
                                              x1T[:, c, :])
                        else:
                            nc.vector.tensor_tensor(x1T[:, c, :], pss[j],
                                                    x1T[:, c, :], ALU_add)
        p_xn2.release()

        p_x1.release()
        p_head.release()
        kv_out.release()
        consts.release()

    nc.compile()
    return nc


def get_program():
    if "nc" not in _CACHE:
        _CACHE["nc"] = _build_program()
    return _CACHE["nc"]


def make_in_maps(x, scale_attn, scale_ffn, Wq, bq, Wk, bk, Wv, bv, Wo, bo,
                 W1, b1, W2, b2):
    """Host-side prep: fold rmsnorm scales into weight rows, convert matmul
    operands to bf16, build per-core rotated+transposed x."""
    import ml_dtypes

    f = np.float32
    bf = ml_dtypes.bfloat16
    sa = np.asarray(scale_attn, f)[:, None]
    sf = np.asarray(scale_ffn, f)[:, None]
    wq_s = np.ascontiguousarray((np.asarray(Wq, f) * sa).astype(bf))
    wk_s = np.ascontiguousarray((np.asarray(Wk, f) * sa).astype(bf))
    wv_s = np.ascontiguousarray((np.asarray(Wv, f) * sa).astype(bf))
    w1_s = np.ascontiguousarray((np.asarray(W1, f) * sf).astype(bf))
    wo_c = np.ascontiguousarray(np.asarray(Wo, f).astype(bf))
    w2_c = np.ascontiguousarray(np.asarray(W2, f).astype(bf))
    shared = dict(
        wq=wq_s, wk=wk_s, wv=wv_s, wo=wo_c, w1=w1_s, w2=w2_c,
        bq=np.asarray(bq, f), bk=np.asarray(bk, f), bv=np.asarray(bv, f),
        bo=np.asarray(bo, f), b1=np.asarray(b1, f), b2=np.asarray(b2, f),
    )
    x = np.asarray(x, f).astype(bf)
    in_maps = []
    for c in range(NCORES):
        be, r0 = c // 4, (c % 4) * T
        xb = x[be]
        x_rot_t = np.ascontiguousarray(np.roll(xb, -r0, axis=0).T)
        m = dict(shared)
        m["x_t"] = x_rot_t
        in_maps.append(m)
    return in_maps


def kernel(**inputs):
    global LAST_RESULTS
    from concourse import bass_utils

    nc = get_program()
    in_maps = make_in_maps(**inputs)
    res = bass_utils.run_bass_kernel_spmd(nc, in_maps, core_ids=list(range(NCORES)))
    LAST_RESULTS = res
    x = np.asarray(inputs["x"], np.float32)
    out = np.empty_like(x)
    for c in range(NCORES):
        be, r0 = c // 4, (c % 4) * T
        out[be, r0:r0 + T, :] = np.asarray(res.results[c]["out_t"]).T
    return out


# revision 34
# speedup vs baseline: 1.2017x; 1.2017x over previous
"""Trainium2 Bass kernel for a pre-norm MQA decoder layer (dense_transformer).

Model (per batch element b, seq s=2048, d=4096, 32 heads x dk=128, d_ff=16384):
  xn = rmsnorm(x)*scale_attn; q,k,v = proj(xn) (MQA: single k/v head)
  attn = softmax(q k^T / sqrt(dk)) v;  x1 = x + attn @ Wo + bo
  xn2 = rmsnorm(x1)*scale_ffn;  out = x1 + gelu(xn2 @ W1 + b1) @ W2 + b2

Sharding: pure data parallel over 8 cores. Each core owns 512 query tokens
(batch be=c//4, rows (c%4)*512..+512) and redundantly computes the full
2048-token K/V for its batch element (cheap for MQA: dk=128). No collectives.
Per-core kv input is rotated so the core's own 512 tokens are always slab 0
(softmax is permutation-invariant over kv).

Everything on device is feature-major [d on partitions, tokens on free]: the
host ships x pre-transposed ([D, S] bf16) and transposes the [D, T] f32
output back, so the kernel has no PE transposes except 16 tiny V-tiles.
Per-token rmsnorm stats are partition reductions done as ones-vector matmuls,
then broadcast back with a K=1 matmul; reciprocals use the fast approx DVE op.
All matmul operands are bf16 (separate pulled-ahead LDWEIGHTS + FWL; f32/f32r
would self-load the stationary inside MATMUL at ~2.2x the cost). Accumulation
is f32 in PSUM; the residual stream x1 is f32 in SBUF.

Engines execute in program order, so the program is laid out to keep the PE
dense (HAM re-throttles to 1.2GHz after ~3.4us of idle):
 - Q-projection m-groups are interleaved between the four K/V token groups,
   covering each group's DMA/normalize latency with 27us of Q matmuls.
 - Attention issues score matmuls one kv-pair ahead and exponentiates two
   512-wide chunks per Activation instruction, so the sum/attn matmuls never
   wait on the scalar engine.
 - GEMM sections get 6-8 PSUM banks so consecutive accumulator blocks
   overlap; cheap DVE work (x1 init, rmsnorm2 stats) is interleaved where the
   PE would otherwise wait.
"""

import sys

if "/opt/trn_rl_repo" not in sys.path:
    sys.path.insert(0, "/opt/trn_rl_repo")

import numpy as np

P = 128
T = 512            # tokens per core
D = 4096
DC = D // P        # 32 feature chunks
DK = 128
NH = 32
S = 2048           # kv length
SC = S // P        # 16 kv chunks
SP = SC // 2       # kv chunk pairs
DFF = 16384
FC = DFF // P      # 128 ff chunks
FBLK = 1024        # FFN f-block width
NFB = DFF // FBLK  # 16 f-blocks
NCORES = 8
EPS = 1e-10
KSCALE = 1.0 / float(np.sqrt(128.0))

_CACHE = {}
LAST_RESULTS = None  # test.py reads exec_time_ns from here


def _build_program():
    import concourse.tile as tile
    from concourse import bacc, mybir
    from concourse.masks import make_identity

    f32 = mybir.dt.float32
    bf16 = mybir.dt.bfloat16
    AF = mybir.ActivationFunctionType
    ALU = mybir.AluOpType

    nc = bacc.Bacc("TRN2", target_bir_lowering=False, num_devices=NCORES)

    x_t = nc.dram_tensor("x_t", [D, S], bf16, kind="ExternalInput")
    wq = nc.dram_tensor("wq", [D, D], bf16, kind="ExternalInput")
    wk = nc.dram_tensor("wk", [D, DK], bf16, kind="ExternalInput")
    wv = nc.dram_tensor("wv", [D, DK], bf16, kind="ExternalInput")
    wo = nc.dram_tensor("wo", [D, D], bf16, kind="ExternalInput")
    w1 = nc.dram_tensor("w1", [D, DFF], bf16, kind="ExternalInput")
    w2 = nc.dram_tensor("w2", [DFF, D], bf16, kind="ExternalInput")
    bq = nc.dram_tensor("bq", [D], f32, kind="ExternalInput")
    bk = nc.dram_tensor("bk", [DK], f32, kind="ExternalInput")
    bv = nc.dram_tensor("bv", [DK], f32, kind="ExternalInput")
    bo = nc.dram_tensor("bo", [D], f32, kind="ExternalInput")
    b1 = nc.dram_tensor("b1", [DFF], f32, kind="ExternalInput")
    b2 = nc.dram_tensor("b2", [D], f32, kind="ExternalInput")
    out_t = nc.dram_tensor("out_t", [D, T], f32, kind="ExternalOutput")

    lowp = nc.allow_low_precision(
        reason="bf16 matmul operands are the intended precision here")
    with lowp, tile.TileContext(nc) as tc:
        consts = tc.alloc_tile_pool(name="consts", bufs=1)
        ident = consts.tile([P, P], f32)
        make_identity(nc, ident)
        ident_b = consts.tile([P, P], bf16)
        nc.vector.tensor_copy(ident_b, ident)
        ones_f = consts.tile([P, 1], f32)
        nc.vector.memset(ones_f, 1.0)
        ones_col = consts.tile([P, 1], bf16)
        nc.vector.tensor_copy(ones_col, ones_f)
        ones_rf = consts.tile([1, P], f32)
        nc.vector.memset(ones_rf, 1.0)
        ones_row = consts.tile([1, P], bf16)
        nc.vector.tensor_copy(ones_row, ones_rf)
        eps_sb = consts.tile([P, 1], f32)
        nc.vector.memset(eps_sb, EPS)
        bq_sb = consts.tile([P, DC], f32)
        nc.sync.dma_start(bq_sb, bq[:].rearrange("(c p) -> p c", p=P))
        bo_sb = consts.tile([P, DC], f32)
        nc.sync.dma_start(bo_sb, bo[:].rearrange("(c p) -> p c", p=P))
        b2_sb = consts.tile([P, DC], f32)
        nc.sync.dma_start(b2_sb, b2[:].rearrange("(c p) -> p c", p=P))
        b1_sb = consts.tile([P, FC], f32)
        nc.sync.dma_start(b1_sb, b1[:].rearrange("(c p) -> p c", p=P))
        bk_sb = consts.tile([P, 1], f32)
        nc.sync.dma_start(bk_sb, bk[:][:, None])
        bv_sb = consts.tile([P, 1], f32)
        nc.sync.dma_start(bv_sb, bv[:][:, None])

        nb2 = consts.tile([P, T], bf16)

        # long-lived activation buffers (never released; FFN fits alongside)
        kv_out = tc.alloc_tile_pool(name="kv_out", bufs=1)
        kT = kv_out.tile([P, S], bf16)          # k^T: dk on partitions
        vtok = kv_out.tile([P, SC, DK], bf16)   # v token-major kv chunks
        p_xn = tc.alloc_tile_pool(name="p_xn", bufs=1)
        xnT = p_xn.tile([P, DC, T], bf16)
        p_head = tc.alloc_tile_pool(name="p_head", bufs=32)

        # ---- Phase 1+2: K/V token groups with Q-projection m-groups
        # interleaved to keep the PE dense while group DMAs/norms land.
        q_tiles = []
        with (
            tc.tile_pool(name="gchunk", bufs=44) as gchunk_p,
            tc.tile_pool(name="sqp", bufs=2) as sq_p,
            tc.tile_pool(name="wkv", bufs=1) as wkv_p,
            tc.tile_pool(name="nstat", bufs=3) as nstat_p,
            tc.tile_pool(name="nbp", bufs=2) as nb_p,
            tc.tile_pool(name="vtmp", bufs=2) as vtmp_p,
            tc.tile_pool(name="wq_s", bufs=48) as wq_p,
            tc.tile_pool(name="ps_misc", bufs=2, space="PSUM") as ps_misc,
            tc.tile_pool(name="ps_kv", bufs=2, space="PSUM") as ps_kv,
            tc.tile_pool(name="ps_tr", bufs=1, space="PSUM") as ps_tr,
            tc.tile_pool(name="ps_q", bufs=3, space="PSUM") as ps_q,
        ):
            # K/V weights: load once, reuse across all 4 token groups
            wk_sb = wkv_p.tile([P, DC, DK], bf16)
            wv_sb = wkv_p.tile([P, DC, DK], bf16)
            nc.sync.dma_start(wk_sb, wk[:].rearrange("(c p) k -> p c k", p=P))
            nc.sync.dma_start(wv_sb, wv[:].rearrange("(c p) k -> p c k", p=P))

            def do_group(g):
                chunks = []
                ssum = ps_misc.tile([1, T], f32, tag="misc", name=f"ss{g}")
                for c in range(DC):
                    if g == 0:
                        ch = xnT[:, c, :]
                    else:
                        ch = gchunk_p.tile([P, T], bf16, tag="ch")
                    nc.sync.dma_start(
                        ch, x_t[c * P:(c + 1) * P, g * T:(g + 1) * T])
                    chunks.append(ch)
                    sq = sq_p.tile([P, T], bf16, tag="sq")
                    nc.vector.tensor_mul(sq, ch, ch)
                    nc.tensor.matmul(ssum, ones_col, sq,
                                     start=(c == 0), stop=(c == DC - 1))
                rms = nstat_p.tile([1, T], f32, tag="rms")
                nc.scalar.activation(rms, ssum, AF.Sqrt, bias=eps_sb[:1, 0:1],
                                     scale=1.0 / D)
                inv32 = nstat_p.tile([1, T], f32, tag="inv32")
                nc.vector.reciprocal_approx_fast(out=inv32, in_=rms)
                inv16 = nstat_p.tile([1, T], bf16, tag="inv16")
                nc.vector.tensor_copy(inv16, inv32)
                nbg_ps = ps_misc.tile([P, T], f32, tag="misc", name=f"nb{g}")
                nc.tensor.matmul(nbg_ps, ones_row, inv16, start=True, stop=True)
                nbg = nb_p.tile([P, T], bf16, tag="nbg")
                nc.vector.tensor_copy(nbg, nbg_ps)

                kps = ps_kv.tile([P, T], f32, tag="kv", name=f"kps{g}")
                vps = ps_kv.tile([P, T], f32, tag="kv", name=f"vps{g}")
                for c in range(DC):
                    nc.vector.tensor_mul(chunks[c], chunks[c], nbg)
                    nc.tensor.matmul(kps, wk_sb[:, c, :], chunks[c],
                                     start=(c == 0), stop=(c == DC - 1))
                    nc.tensor.matmul(vps, wv_sb[:, c, :], chunks[c],
                                     start=(c == 0), stop=(c == DC - 1))
                nc.scalar.activation(kT[:, g * T:(g + 1) * T], kps, AF.Identity,
                                     bias=bk_sb[:, 0:1])
                vt = vtmp_p.tile([P, T], bf16, tag="vt")
                nc.scalar.activation(vt, vps, AF.Identity, bias=bv_sb[:, 0:1])
                for q4 in range(4):
                    pt = ps_tr.tile([P, P], bf16, tag="tr")
                    nc.tensor.transpose(pt, vt[:, q4 * P:(q4 + 1) * P], ident_b)
                    nc.vector.tensor_copy(vtok[:, g * 4 + q4, :], pt)

            def do_qmg(mg):
                # j-serialized: one PSUM accumulator at a time, the 32 weight
                # tiles of this m-group stay resident across the 4 j-passes
                wbs = []
                for kc in range(DC):
                    wb = wq_p.tile([P, 512], bf16, tag="wq")
                    nc.sync.dma_start(
                        wb, wq[kc * P:(kc + 1) * P, mg * 512:(mg + 1) * 512])
                    wbs.append(wb)
                for j in range(4):
                    m = mg * 4 + j
                    psq = ps_q.tile([P, T], f32, tag="q", name=f"psq{m}")
                    for kc in range(DC):
                        nc.tensor.matmul(psq, wbs[kc][:, j * P:(j + 1) * P],
                                         xnT[:, kc, :],
                                         start=(kc == 0), stop=(kc == DC - 1))
                    qt = p_head.tile([P, T], bf16, tag="head", name=f"q{m}")
                    nc.scalar.activation(qt, psq, AF.Identity,
                                         bias=bq_sb[:, m:m + 1])
                    q_tiles.append(qt)

            do_group(0)
            do_qmg(0)
            do_qmg(1)
            do_group(1)
            do_qmg(2)
            do_qmg(3)
            do_group(2)
            do_qmg(4)
            do_qmg(5)
            do_group(3)
            do_qmg(6)
            do_qmg(7)

        # x1T: f32 residual-stream accumulator (allocated only now; phase 1+2
        # needed the SBUF for resident weight tiles)
        p_x1 = tc.alloc_tile_pool(name="p_x1", bufs=1)
        x1T = p_x1.tile([P, DC, T], f32)

        # ---- Phase 3+4: attention; x1T init (raw own x^T + bo) interleaved
        # one chunk per head on DMA+DVE. Attention output for head h
        # overwrites q_tiles[h] in place. Scores run one kv-pair ahead and
        # exp handles two 512-chunks per instruction.
        with (
            tc.tile_pool(name="xrp", bufs=4) as xr_p,
            tc.tile_pool(name="expp", bufs=4) as exp_p,
            tc.tile_pool(name="bcp", bufs=2) as bc_p,
            tc.tile_pool(name="smalls", bufs=4) as small_p,
            tc.tile_pool(name="ps_scp", bufs=2, space="PSUM") as ps_scp,
            tc.tile_pool(name="ps_sum", bufs=2, space="PSUM") as ps_sum,
            tc.tile_pool(name="ps_at", bufs=2, space="PSUM") as ps_at,
        ):
            for h in range(NH):
                # x1T chunk h init: raw own x^T + bo (no PE work)
                xr = xr_p.tile([P, T], bf16, tag="xr", name=f"xr{h}")
                nc.sync.dma_start(xr, x_t[h * P:(h + 1) * P, 0:T])
                nc.vector.tensor_scalar_add(x1T[:, h, :], xr, bo_sb[:, h:h + 1])

                sum_ps = ps_sum.tile([1, T], f32, tag="sum", name=f"sum{h}")
                at_ps = ps_at.tile([P, T], f32, tag="at", name=f"at{h}")
                ex_pairs = []

                def issue_scores(pp):
                    # score pairs run two pairs ahead of sum/at so the PE
                    # never waits on the scalar engine's exp; one exp covers
                    # both 512-chunks (fewer ACT instructions + sem waits)
                    pr = ps_scp.tile([P, 2, T], f32, tag="sc",
                                     name=f"sc{h}_{pp}")
                    nc.tensor.matmul(pr[:, 0, :],
                                     kT[:, (2 * pp) * P:(2 * pp + 1) * P],
                                     q_tiles[h], start=True, stop=True)
                    nc.tensor.matmul(pr[:, 1, :],
                                     kT[:, (2 * pp + 1) * P:(2 * pp + 2) * P],
                                     q_tiles[h], start=True, stop=True)
                    ex = exp_p.tile([P, 2, T], bf16, tag="ex",
                                    name=f"ex{h}_{pp}")
                    nc.scalar.activation(ex, pr, AF.Exp, scale=KSCALE)
                    ex_pairs.append(ex)

                issue_scores(0)
                issue_scores(1)
                for pp in range(SP):
                    ex = ex_pairs[pp]
                    for half in range(2):
                        sc = 2 * pp + half
                        nc.tensor.matmul(sum_ps, ones_col, ex[:, half, :],
                                         start=(sc == 0), stop=(sc == SC - 1))
                        nc.tensor.matmul(at_ps, vtok[:, sc, :], ex[:, half, :],
                                         start=(sc == 0), stop=(sc == SC - 1))
                    if pp + 2 < SP:
                        issue_scores(pp + 2)
                rec32 = small_p.tile([1, T], f32, tag="rec32", name=f"rec32_{h}")
                nc.vector.reciprocal_approx_fast(out=rec32, in_=sum_ps)
                rec = small_p.tile([1, T], bf16, tag="rec", name=f"rec{h}")
                nc.vector.tensor_copy(rec, rec32)
                bc_ps = ps_sum.tile([P, T], f32, tag="sum", name=f"bc{h}")
                nc.tensor.matmul(bc_ps, ones_row, rec, start=True, stop=True)
                bc = bc_p.tile([P, T], f32, tag="bc", name=f"bcs{h}")
                nc.vector.tensor_copy(bc, bc_ps)
                nc.vector.tensor_mul(q_tiles[h], at_ps, bc)
        attn_tiles = q_tiles

        # ---- Phase 5: Wo + residual into x1T; rmsnorm2 stats (sq + ones-
        # matmul) interleaved per jg block right after each chunk finalizes.
        with (
            tc.tile_pool(name="wo_s", bufs=8) as wo_p,
            tc.tile_pool(name="sq2", bufs=3) as sq2_p,
            tc.tile_pool(name="smalls2", bufs=2) as small2_p,
            tc.tile_pool(name="ps_wo", bufs=6, space="PSUM") as ps_wo,
            tc.tile_pool(name="ps_ss2", bufs=1, space="PSUM") as ps_ss2,
            tc.tile_pool(name="ps_nb2", bufs=1, space="PSUM") as ps_nb2,
        ):
            ssum2 = ps_ss2.tile([1, T], f32, tag="ss2")
            for jg in range(8):
                pss = [ps_wo.tile([P, T], f32, tag="wo", name=f"pswo{jg}_{j}")
                       for j in range(4)]
                for kc in range(DC):
                    wb = wo_p.tile([P, 512], bf16, tag="wob")
                    nc.sync.dma_start(wb, wo[kc * P:(kc + 1) * P, jg * 512:(jg + 1) * 512])
                    for j in range(4):
                        nc.tensor.matmul(pss[j], wb[:, j * P:(j + 1) * P],
                                         attn_tiles[kc],
                                         start=(kc == 0), stop=(kc == DC - 1))
                for j in range(4):
                    c = jg * 4 + j
                    nc.vector.tensor_tensor(x1T[:, c, :], pss[j], x1T[:, c, :], ALU.add)
                    sq = sq2_p.tile([P, T], bf16, tag="sq2", name=f"sq2_{c}")
                    nc.vector.tensor_mul(sq, x1T[:, c, :], x1T[:, c, :])
                    nc.tensor.matmul(ssum2, ones_col, sq,
                                     start=(c == 0), stop=(c == DC - 1))
            rms2 = small2_p.tile([1, T], f32, tag="rms2")
            nc.scalar.activation(rms2, ssum2, AF.Sqrt, bias=eps_sb[:1, 0:1],
                                 scale=1.0 / D)
            inv2 = small2_p.tile([1, T], f32, tag="inv2")
            nc.vector.reciprocal_approx_fast(out=inv2, in_=rms2)
            inv2b = small2_p.tile([1, T], bf16, tag="inv2b")
            nc.vector.tensor_copy(inv2b, inv2)
            nb_ps = ps_nb2.tile([P, T], f32, tag="nb2")
            nc.tensor.matmul(nb_ps, ones_row, inv2b, start=True, stop=True)
            nc.vector.tensor_copy(nb2, nb_ps)

        # ---- Phase 6: FFN, f-blocked; W2 accumulated into x1T in place.
        # Last f-block fuses +b2 and streams the finished chunk out to DRAM.
        p_xn2 = tc.alloc_tile_pool(name="p_xn2", bufs=1)
        xn2T = p_xn2.tile([P, DC, T], bf16)
        for c in range(DC):
            nc.vector.tensor_mul(xn2T[:, c, :], x1T[:, c, :], nb2)

        MGS = FBLK // 512          # m-groups per f-block
        FCB = FBLK // P            # f chunks per f-block
        ALU_add = ALU.add
        with (
            tc.tile_pool(name="wf_s", bufs=8) as wf_p,
            tc.tile_pool(name="htp", bufs=20) as ht_p,
            tc.tile_pool(name="ps_ffn", bufs=8, space="PSUM") as ps_ffn,
        ):
            for fb in range(NFB):
                last_fb = fb == NFB - 1
                ht_tiles = []
                for mg in range(MGS):
                    pss = [ps_ffn.tile([P, T], f32, tag="ffn", name=f"psw1_{fb}_{mg}_{j}")
                           for j in range(4)]
                    for kc in range(DC):
                        wb = wf_p.tile([P, 512], bf16, tag="wf")
                        nc.sync.dma_start(
                            wb, w1[kc * P:(kc + 1) * P,
                                   fb * FBLK + mg * 512:fb * FBLK + (mg + 1) * 512])
                        for j in range(4):
                            nc.tensor.matmul(pss[j], wb[:, j * P:(j + 1) * P],
                                             xn2T[:, kc, :],
                                             start=(kc == 0), stop=(kc == DC - 1))
                    for j in range(4):
                        m = fb * FCB + mg * 4 + j
                        ht = ht_p.tile([P, T], bf16, tag="ht", name=f"ht{m}")
                        nc.scalar.activation(ht, pss[j], AF.Gelu, bias=b1_sb[:, m:m + 1])
                        ht_tiles.append(ht)
                for jg in range(8):
                    pss = [ps_ffn.tile([P, T], f32, tag="ffn", name=f"psw2_{fb}_{jg}_{j}")
                           for j in range(4)]
                    for fc in range(FCB):
                        wb = wf_p.tile([P, 512], bf16, tag="wf")
                        nc.sync.dma_start(
                            wb, w2[fb * FBLK + fc * P:fb * FBLK + (fc + 1) * P,
                                   jg * 512:(jg + 1) * 512])
                        for j in range(4):
                            nc.tensor.matmul(pss[j], wb[:, j * P:(j + 1) * P],
                                             ht_tiles[fc],
                                             start=(fc == 0), stop=(fc == FCB - 1))
                    for j in range(4):
                        c = jg * 4 + j
                        if last_fb:
                            # x1 += pss + b2, then stream the chunk out
                            nc.vector.scalar_tensor_tensor(
                                out=x1T[:, c, :], in0=pss[j],
                                scalar=b2_sb[:, c:c + 1], in1=x1T[:, c, :],
                                op0=ALU_add, op1=ALU_add)
                            nc.sync.dma_start(out_t[c * P:(c + 1) * P, :],
                                              x1T[:, c, :])
                        else:
                            nc.vector.tensor_tensor(x1T[:, c, :], pss[j],
                                                    x1T[:, c, :], ALU_add)
        p_xn2.release()

        p_x1.release()
        p_head.release()
        p_xn.release()
        kv_out.release()
        consts.release()

    nc.compile()
    return nc


def get_program():
    if "nc" not in _CACHE:
        _CACHE["nc"] = _build_program()
    return _CACHE["nc"]


def make_in_maps(x, scale_attn, scale_ffn, Wq, bq, Wk, bk, Wv, bv, Wo, bo,
                 W1, b1, W2, b2):
    """Host-side prep: fold rmsnorm scales into weight rows, convert matmul
    operands to bf16, build per-core rotated+transposed x."""
    import ml_dtypes

    f = np.float32
    bf = ml_dtypes.bfloat16
    sa = np.asarray(scale_attn, f)[:, None]
    sf = np.asarray(scale_ffn, f)[:, None]
    wq_s = np.ascontiguousarray((np.asarray(Wq, f) * sa).astype(bf))
    wk_s = np.ascontiguousarray((np.asarray(Wk, f) * sa).astype(bf))
    wv_s = np.ascontiguousarray((np.asarray(Wv, f) * sa).astype(bf))
    w1_s = np.ascontiguousarray((np.asarray(W1, f) * sf).astype(bf))
    wo_c = np.ascontiguousarray(np.asarray(Wo, f).astype(bf))
    w2_c = np.ascontiguousarray(np.asarray(W2, f).astype(bf))
    shared = dict(
        wq=wq_s, wk=wk_s, wv=wv_s, wo=wo_c, w1=w1_s, w2=w2_c,
        bq=np.asarray(bq, f), bk=np.asarray(bk, f), bv=np.asarray(bv, f),
        bo=np.asarray(bo, f), b1=np.asarray(b1, f), b2=np.asarray(b2, f),
    )
    x = np.asarray(x, f).astype(bf)
    in_maps = []
    for c in range(NCORES):
        be, r0 = c // 4, (c % 4) * T
        xb = x[be]
        x_rot_t = np.ascontiguousarray(np.roll(xb, -r0, axis=0).T)
        m = dict(shared)
        m["x_t"] = x_rot_t
        in_maps.append(m)
    return in_maps


def kernel(**inputs):
    global LAST_RESULTS
    from concourse import bass_utils

    nc = get_program()
    in_maps = make_in_maps(**inputs)
    res = bass_utils.run_bass_kernel_spmd(nc, in_maps, core_ids=list(range(NCORES)))
    LAST_RESULTS = res
    x = np.asarray(inputs["x"], np.float32)
    out = np.empty_like(x)
    for c in range(NCORES):
        be, r0 = c // 4, (c % 4) * T
        out[be, r0:r0 + T, :] = np.asarray(res.results[c]["out_t"]).T
    return out
